# revision 9
# baseline (speedup 1.0000x reference)
"""Causal self-attention (B=2, T=2048, C=1024, H=16, D=64) on 8 trn2 cores.

Sharding: core = b*4 + hg  (data-parallel over batch b, tensor-parallel over
4 head-groups of 4 heads).  Each core computes q/k/v projections for its
256 head-dims, flash-style causal attention for its 4 heads, and a partial
output projection (its 256-column slice of Wp).  Partials are summed on the
host (the all-reduce), bias bp added there too.

Device layout notes:
  - All matmuls run in float32r (TRN2 reduced-precision fp32: 11-bit
    mantissa, 4x the fp32 matmul throughput).
  - qT/kT stored as [d', t] with 2 heads packed per 128 partitions.
  - v stored naturally [t, d'] with a ones-column appended per head
    (65 cols/head) so the PV matmul's output row 64 is the softmax
    denominator l[q] for free.
  - S^T tiles [k=128, q=512] land in PSUM, exp on ACT (scale=1/8 fused,
    no max-subtraction: |S|<~8 so exp is safe in fp32).  Diagonal chunks
    are column-restricted to the causal range; only the 128-wide triangle
    needs a mask multiply.
  - The attention loop is software-pipelined: S matmuls for group g+1 are
    emitted before PV of group g, so the PE never waits on ACT's exp and
    the HAM clock gate stays warm.
  - Normalization: l -> PE partition-broadcast -> reciprocal_approx_fast
    -> DVE mul into yT.
"""
import numpy as np
from contextlib import ExitStack

B, T, C, H, D = 2, 2048, 1024, 16, 64
HLOC = 4            # heads per core
CLOC = HLOC * D     # 256 head-dims per core
VW = HLOC * 65      # v width with ones-columns: 260
N_CORES = 8
TQ = 512            # q tile width
KC = 128            # k chunk
NCC = C // 128      # 8 contraction chunks

MM_DT = "float32r"  # or "float32" (4x slower, exact)

_CACHE = {}


def build_nc(with_qk_bias=True):
    import concourse.tile as tile
    from concourse import bacc, mybir

    f32 = mybir.dt.float32
    fmm = getattr(mybir.dt, MM_DT)
    Exp = mybir.ActivationFunctionType.Exp

    nc = bacc.Bacc("TRN2", target_bir_lowering=False, debug=False,
                   num_devices=N_CORES)
    xT = nc.dram_tensor("xT", [C, T], fmm, kind="ExternalInput").ap()
    wqT = nc.dram_tensor("wqT", [C, CLOC], fmm, kind="ExternalInput").ap()
    wkT = nc.dram_tensor("wkT", [C, CLOC], fmm, kind="ExternalInput").ap()
    wvT = nc.dram_tensor("wvT", [C, VW], fmm, kind="ExternalInput").ap()
    wpT = nc.dram_tensor("wpT", [CLOC, C], fmm, kind="ExternalInput").ap()
    bq = nc.dram_tensor("bq", [1, CLOC], fmm, kind="ExternalInput").ap()
    bk = nc.dram_tensor("bk", [1, CLOC], fmm, kind="ExternalInput").ap()
    bv = nc.dram_tensor("bv", [1, VW], fmm, kind="ExternalInput").ap()
    ones = nc.dram_tensor("ones", [1, TQ], fmm, kind="ExternalInput").ap()
    tri = nc.dram_tensor("tri", [KC, KC], fmm, kind="ExternalInput").ap()
    po = nc.dram_tensor("po", [T, C], f32, kind="ExternalOutput").ap()

    with tile.TileContext(nc) as tc, ExitStack() as ctx:
        persist = ctx.enter_context(tc.tile_pool(name="persist", bufs=1))
        pt_pool = ctx.enter_context(tc.tile_pool(name="pt", bufs=3))
        stage = ctx.enter_context(tc.tile_pool(name="stage", bufs=2))
        norm_pool = ctx.enter_context(tc.tile_pool(name="norm", bufs=3))
        ps_small = ctx.enter_context(
            tc.tile_pool(name="ps_small", bufs=2, space="PSUM"))
        ps_st = ctx.enter_context(
            tc.tile_pool(name="ps_st", bufs=2, space="PSUM"))
        ps_yt = ctx.enter_context(
            tc.tile_pool(name="ps_yt", bufs=2, space="PSUM"))

        # ---- persistent SBUF tensors + loads ----
        xT_sb = [persist.tile([128, T], fmm, tag=f"xT{c}", name=f"xT{c}") for c in range(NCC)]
        wq_sb = [persist.tile([128, CLOC], fmm, tag=f"wq{c}", name=f"wq{c}") for c in range(NCC)]
        wk_sb = [persist.tile([128, CLOC], fmm, tag=f"wk{c}", name=f"wk{c}") for c in range(NCC)]
        wv_sb = [persist.tile([128, VW], fmm, tag=f"wv{c}", name=f"wv{c}") for c in range(NCC)]
        wp_sb = [persist.tile([128, C], fmm, tag=f"wp{m}", name=f"wp{m}") for m in range(2)]
        bq_sb = persist.tile([1, CLOC], fmm, tag="bq")
        bk_sb = persist.tile([1, CLOC], fmm, tag="bk")
        bv_sb = persist.tile([1, VW], fmm, tag="bv")
        ones_sb = persist.tile([1, TQ], fmm, tag="ones")
        tri_sb = persist.tile([KC, KC], fmm, tag="tri")
        qT_sb = [persist.tile([128, T], fmm, tag=f"qT{m}", name=f"qT{m}") for m in range(2)]
        kT_sb = [persist.tile([128, T], fmm, tag=f"kT{m}", name=f"kT{m}") for m in range(2)]
        v_sb = [persist.tile([128, VW], fmm, tag=f"v{t}", name=f"v{t}") for t in range(T // 128)]
        yT_sb = [persist.tile([128, T], fmm, tag=f"yT{m}", name=f"yT{m}") for m in range(2)]

        nc.sync.dma_start(ones_sb[:], ones[:])
        nc.sync.dma_start(tri_sb[:], tri[:])
        nc.sync.dma_start(bq_sb[:], bq[:])
        nc.sync.dma_start(bk_sb[:], bk[:])
        nc.sync.dma_start(bv_sb[:], bv[:])
        for c in range(NCC):
            sl = slice(c * 128, (c + 1) * 128)
            nc.sync.dma_start(xT_sb[c][:, 0:TQ], xT[sl, 0:TQ])
            nc.sync.dma_start(wq_sb[c][:], wqT[sl, :])
        for c in range(NCC):
            sl = slice(c * 128, (c + 1) * 128)
            nc.sync.dma_start(wk_sb[c][:], wkT[sl, :])
            nc.sync.dma_start(wv_sb[c][:], wvT[sl, :])
        for t in range(1, T // TQ):
            tsl = slice(t * TQ, (t + 1) * TQ)
            for c in range(NCC):
                sl = slice(c * 128, (c + 1) * 128)
                nc.sync.dma_start(xT_sb[c][:, tsl], xT[sl, tsl])
        for m in range(2):
            nc.sync.dma_start(wp_sb[m][:], wpT[m * 128:(m + 1) * 128, :])

        # ---- interleaved emission: projections / attention / out-proj ----
        # The PE executes its queue in order, so emission order controls PE
        # density.  Attention for q-tile j only needs projections up to
        # t=j, so projections for t=j+1 and the out-projection for j-1 are
        # woven between attention groups of j to fill PE idle slots (keeps
        # the HAM clock-gate warm).
        def proj_qk(w_sb, b_sb, dst, m, t):
            tsl = slice(t * TQ, (t + 1) * TQ)
            msl = slice(m * 128, (m + 1) * 128)
            ps = ps_small.tile([128, TQ], f32, tag="ps_small")
            for c in range(NCC):
                nc.tensor.matmul(ps[:], w_sb[c][:, msl], xT_sb[c][:, tsl],
                                 start=(c == 0),
                                 stop=(c == NCC - 1 and not with_qk_bias))
            if with_qk_bias:
                nc.tensor.matmul(ps[:], b_sb[0:1, msl], ones_sb[0:1, :],
                                 start=False, stop=True)
            nc.vector.tensor_copy(dst[m][:, tsl], ps[:])

        def proj_v(tt):
            ttsl = slice(tt * 128, tt * 128 + 128)
            ps = ps_small.tile([128, VW], f32, tag="ps_small")
            for c in range(NCC):
                nc.tensor.matmul(ps[:], xT_sb[c][:, ttsl], wv_sb[c][:],
                                 start=(c == 0), stop=False)
            # always emitted: supplies the ones-columns (+ v bias)
            nc.tensor.matmul(ps[:], ones_sb[0:1, 0:128], bv_sb[:],
                             start=False, stop=True)
            nc.vector.tensor_copy(v_sb[tt][:], ps[:])

        def proj_pieces(t):
            out = []
            for w_sb, b_sb, dst in ((wq_sb, bq_sb, qT_sb), (wk_sb, bk_sb, kT_sb)):
                for m in range(2):
                    out.append(lambda w=w_sb, b=b_sb, d=dst, mm=m:
                               proj_qk(w, b, d, mm, t))
            for tt in range(t * 4, t * 4 + 4):
                out.append(lambda x=tt: proj_v(x))
            return out

        def outproj_piece(tt, do):
            ttsl = slice(tt * 128, (tt + 1) * 128)
            dsl = slice(do * TQ, (do + 1) * TQ)
            ops = ps_small.tile([128, TQ], f32, tag="ps_small")
            for m2 in range(2):
                nc.tensor.matmul(ops[:], yT_sb[m2][:, ttsl],
                                 wp_sb[m2][:, dsl],
                                 start=(m2 == 0), stop=(m2 == 1))
            so = stage.tile([128, TQ], f32, tag="so")
            nc.vector.tensor_copy(so[:], ops[:])
            nc.sync.dma_start(po[ttsl, dsl], so[:])

        def outproj_pieces(j):
            return [lambda t=tt, d=do: outproj_piece(t, d)
                    for tt in range(4 * j, 4 * j + 4) for do in range(2)]

        def s_group(j, h, kcs):
            """Emit S matmuls for a k-chunk pair; return (st_tile, info)."""
            m, pr = h // 2, (h % 2) * 64
            st = ps_st.tile([128, 1024], f32, tag="st")
            info = []
            for i, kc in enumerate(kcs):
                coff = max(0, kc * KC - j * TQ)   # causal column offset
                nc.tensor.matmul(
                    st[:, i * TQ + coff:(i + 1) * TQ],
                    kT_sb[m][pr:pr + 64, kc * KC:(kc + 1) * KC],
                    qT_sb[m][pr:pr + 64, j * TQ + coff:(j + 1) * TQ],
                    start=True, stop=True)
                info.append((i, kc, coff))
            return st, info

        def pv_group(j, h, st, info, yt, nk):
            """exp + triangle mask + PV matmuls for a prepared S group."""
            pt = pt_pool.tile([128, 1024], fmm, tag="pt")
            runs = []
            for i, kc, coff in info:
                lo, hi = i * TQ + coff, (i + 1) * TQ
                if runs and runs[-1][1] == lo:
                    runs[-1][1] = hi
                else:
                    runs.append([lo, hi])
            for lo, hi in runs:
                nc.scalar.activation(pt[:, lo:hi], st[:, lo:hi], Exp, scale=0.125)
            for i, kc, coff in info:
                if kc >= 4 * j:   # diagonal chunk: mask the 128-wide triangle
                    lo = i * TQ + coff
                    nc.vector.tensor_mul(pt[:, lo:lo + KC], pt[:, lo:lo + KC],
                                         tri_sb[:])
            for i, kc, coff in info:
                lo = i * TQ + coff
                nc.tensor.matmul(
                    yt[0:65, coff:TQ] if coff else yt[:],
                    v_sb[kc][:, h * 65:(h + 1) * 65],
                    pt[:, lo:(i + 1) * TQ],
                    start=(kc == 0), stop=(kc == nk - 1))

        def normalize(j, h, yt):
            """yT[h slice, j] = yt[0:64] * broadcast(1/l)."""
            m, pr = h // 2, (h % 2) * 64
            l_sb = norm_pool.tile([1, TQ], fmm, tag="l")
            nc.vector.tensor_copy(l_sb[:], yt[64:65, :])
            bc_ps = ps_small.tile([64, TQ], f32, tag="ps_small")
            nc.tensor.matmul(bc_ps[:], ones_sb[0:1, 0:64], l_sb[:],
                             start=True, stop=True)
            bc_sb = stage.tile([64, TQ], f32, tag="bc")
            nc.vector.reciprocal_approx_fast(bc_sb[:], bc_ps[:])
            nc.vector.tensor_mul(yT_sb[m][pr:pr + 64, j * TQ:(j + 1) * TQ],
                                 yt[0:64, :], bc_sb[:])

        for piece in proj_pieces(0):    # prologue
            piece()

        for j in range(T // TQ):
            nk = 4 * (j + 1)
            groups = []
            for h in range(HLOC):
                for k0 in range(0, nk, 2):
                    groups.append((h, [k for k in (k0, k0 + 1) if k < nk]))
            extras = []
            if j + 1 < T // TQ:
                extras += proj_pieces(j + 1)
            if j >= 1:
                extras += outproj_pieces(j - 1)
            ei = 0           # extras emitted so far
            yts = {}
            pending = None   # (h, st, info) awaiting exp/PV
            done_head = None  # head awaiting normalize
            for gi, (h, kcs) in enumerate(groups):
                if h not in yts:
                    yts[h] = ps_yt.tile([65, TQ], f32, tag="yt",
                                        name=f"yt{j}_{h}")
                st, info = s_group(j, h, kcs)
                if pending is not None:
                    ph, pst, pinfo = pending
                    pv_group(j, ph, pst, pinfo, yts[ph], nk)
                    if ph != h:
                        done_head = ph
                    elif done_head is not None:
                        normalize(j, done_head, yts.pop(done_head))
                        done_head = None
                pending = (h, st, info)
                want = (gi + 1) * len(extras) // len(groups)
                while ei < want:
                    extras[ei]()
                    ei += 1
            ph, pst, pinfo = pending
            pv_group(j, ph, pst, pinfo, yts[ph], nk)
            if done_head is not None:
                normalize(j, done_head, yts.pop(done_head))
            normalize(j, ph, yts.pop(ph))
            while ei < len(extras):
                extras[ei]()
                ei += 1

        for piece in outproj_pieces(T // TQ - 1):   # epilogue
            piece()
    nc.compile()
    return nc


def make_in_maps(x, Wq, bq, Wk, bk, Wv, bv, Wp, bp):
    x = np.asarray(x, np.float32)
    Wq, Wk, Wv, Wp = (np.asarray(w, np.float32) for w in (Wq, Wk, Wv, Wp))
    bq, bk, bv = (np.asarray(b, np.float32) for b in (bq, bk, bv))

    ones = np.ones((1, TQ), np.float32)
    kp = np.arange(KC)[:, None]
    qf = np.arange(KC)[None, :]
    tri = (qf >= kp).astype(np.float32)

    in_maps = []
    for core in range(N_CORES):
        b = core // 4
        hg = core % 4
        rows = slice(hg * CLOC, (hg + 1) * CLOC)
        wv_aug = np.zeros((C, VW), np.float32)
        bv_aug = np.zeros((1, VW), np.float32)
        for h in range(HLOC):
            wsl = slice(hg * CLOC + h * D, hg * CLOC + (h + 1) * D)
            wv_aug[:, h * 65:h * 65 + D] = Wv[wsl, :].T
            bv_aug[0, h * 65:h * 65 + D] = bv[wsl]
            bv_aug[0, h * 65 + D] = 1.0
        in_maps.append({
            "xT": np.ascontiguousarray(x[b].T),
            "wqT": np.ascontiguousarray(Wq[rows, :].T),
            "wkT": np.ascontiguousarray(Wk[rows, :].T),
            "wvT": wv_aug,
            "wpT": np.ascontiguousarray(Wp[:, rows].T),
            "bq": np.ascontiguousarray(bq[rows][None, :]),
            "bk": np.ascontiguousarray(bk[rows][None, :]),
            "bv": bv_aug,
            "ones": ones,
            "tri": tri,
        })
    return in_maps


def kernel(x, Wq, bq, Wk, bk, Wv, bv, Wp, bp):
    from concourse.bass_utils import run_bass_kernel_spmd

    with_qk_bias = bool(np.any(np.asarray(bq)) or np.any(np.asarray(bk)))
    key = ("nc", with_qk_bias)
    if key not in _CACHE:
        _CACHE[key] = build_nc(with_qk_bias)
    nc = _CACHE[key]
    in_maps = make_in_maps(x, Wq, bq, Wk, bk, Wv, bv, Wp, bp)
    res = run_bass_kernel_spmd(nc, in_maps, core_ids=list(range(N_CORES)))
    out = np.zeros((B, T, C), np.float32)
    for core in range(N_CORES):
        out[core // 4] += res.results[core]["po"]
    out += np.asarray(bp, np.float32)[None, None, :]
    return out


# revision 10
# speedup vs baseline: 1.1520x; 1.1520x over previous
"""Causal self-attention (B=2, T=2048, C=1024, H=16, D=64) on 8 trn2 cores.

Sharding: core = b*4 + hg  (data-parallel over batch b, tensor-parallel over
4 head-groups of 4 heads).  Each core computes q/k/v projections for its
256 head-dims, flash-style causal attention for its 4 heads, and a partial
output projection (its 256-column slice of Wp).  Partials are summed on the
host (the all-reduce), bias bp added there too.

Device layout notes:
  - All matmuls run in float32r (TRN2 reduced-precision fp32: 11-bit
    mantissa, 4x the fp32 matmul throughput).
  - qT/kT stored as [d', t] with 2 heads packed per 128 partitions.
  - v stored naturally [t, d'] with a ones-column appended per head
    (65 cols/head) so the PV matmul's output row 64 is the softmax
    denominator l[q] for free.
  - S^T tiles [k=128, q=512] land in PSUM, exp on ACT (scale=1/8 fused,
    no max-subtraction: |S|<~8 so exp is safe in fp32).  Diagonal chunks
    are column-restricted to the causal range; only the 128-wide triangle
    needs a mask multiply.
  - The attention loop is software-pipelined: S matmuls for group g+1 are
    emitted before PV of group g, so the PE never waits on ACT's exp and
    the HAM clock gate stays warm.
  - Normalization: l -> PE partition-broadcast -> reciprocal_approx_fast
    -> DVE mul into yT.
"""
import numpy as np
from contextlib import ExitStack

B, T, C, H, D = 2, 2048, 1024, 16, 64
HLOC = 4            # heads per core
CLOC = HLOC * D     # 256 head-dims per core
VW = HLOC * 65      # v width with ones-columns: 260
N_CORES = 8
TQ = 512            # q tile width
KC = 128            # k chunk
NCC = C // 128      # 8 contraction chunks

MM_DT = "float32r"  # or "float32" (4x slower, exact)

_CACHE = {}


def build_nc(with_qk_bias=True):
    import concourse.tile as tile
    from concourse import bacc, mybir

    f32 = mybir.dt.float32
    fmm = getattr(mybir.dt, MM_DT)
    Exp = mybir.ActivationFunctionType.Exp

    nc = bacc.Bacc("TRN2", target_bir_lowering=False, debug=False,
                   num_devices=N_CORES)
    xT = nc.dram_tensor("xT", [C, T], fmm, kind="ExternalInput").ap()
    wqT = nc.dram_tensor("wqT", [C, CLOC], fmm, kind="ExternalInput").ap()
    wkT = nc.dram_tensor("wkT", [C, CLOC], fmm, kind="ExternalInput").ap()
    wvT = nc.dram_tensor("wvT", [C, VW], fmm, kind="ExternalInput").ap()
    wpT = nc.dram_tensor("wpT", [CLOC, C], fmm, kind="ExternalInput").ap()
    bq = nc.dram_tensor("bq", [1, CLOC], fmm, kind="ExternalInput").ap()
    bk = nc.dram_tensor("bk", [1, CLOC], fmm, kind="ExternalInput").ap()
    bv = nc.dram_tensor("bv", [1, VW], fmm, kind="ExternalInput").ap()
    ones = nc.dram_tensor("ones", [1, TQ], fmm, kind="ExternalInput").ap()
    tri = nc.dram_tensor("tri", [KC, KC], fmm, kind="ExternalInput").ap()
    po = nc.dram_tensor("po", [T, C], f32, kind="ExternalOutput").ap()

    with tile.TileContext(nc) as tc, ExitStack() as ctx:
        persist = ctx.enter_context(tc.tile_pool(name="persist", bufs=1))
        pt_pool = ctx.enter_context(tc.tile_pool(name="pt", bufs=3))
        stage = ctx.enter_context(tc.tile_pool(name="stage", bufs=2))
        norm_pool = ctx.enter_context(tc.tile_pool(name="norm", bufs=3))
        ps_small = ctx.enter_context(
            tc.tile_pool(name="ps_small", bufs=2, space="PSUM"))
        ps_st = ctx.enter_context(
            tc.tile_pool(name="ps_st", bufs=2, space="PSUM"))
        ps_yt = ctx.enter_context(
            tc.tile_pool(name="ps_yt", bufs=2, space="PSUM"))

        # ---- persistent SBUF tensors + loads ----
        xT_sb = [persist.tile([128, T], fmm, tag=f"xT{c}", name=f"xT{c}") for c in range(NCC)]
        wq_sb = [persist.tile([128, CLOC], fmm, tag=f"wq{c}", name=f"wq{c}") for c in range(NCC)]
        wk_sb = [persist.tile([128, CLOC], fmm, tag=f"wk{c}", name=f"wk{c}") for c in range(NCC)]
        wv_sb = [persist.tile([128, VW], fmm, tag=f"wv{c}", name=f"wv{c}") for c in range(NCC)]
        wp_sb = [persist.tile([128, C], fmm, tag=f"wp{m}", name=f"wp{m}") for m in range(2)]
        bq_sb = persist.tile([1, CLOC], fmm, tag="bq")
        bk_sb = persist.tile([1, CLOC], fmm, tag="bk")
        bv_sb = persist.tile([1, VW], fmm, tag="bv")
        ones_sb = persist.tile([1, TQ], fmm, tag="ones")
        tri_sb = persist.tile([KC, KC], fmm, tag="tri")
        qT_sb = [persist.tile([128, T], fmm, tag=f"qT{m}", name=f"qT{m}") for m in range(2)]
        kT_sb = [persist.tile([128, T], fmm, tag=f"kT{m}", name=f"kT{m}") for m in range(2)]
        v_sb = [persist.tile([128, VW], fmm, tag=f"v{t}", name=f"v{t}") for t in range(T // 128)]
        yT_sb = [persist.tile([128, T], fmm, tag=f"yT{m}", name=f"yT{m}") for m in range(2)]

        nc.sync.dma_start(ones_sb[:], ones[:])
        nc.sync.dma_start(tri_sb[:], tri[:])
        nc.sync.dma_start(bq_sb[:], bq[:])
        nc.sync.dma_start(bk_sb[:], bk[:])
        nc.sync.dma_start(bv_sb[:], bv[:])
        for c in range(NCC):
            sl = slice(c * 128, (c + 1) * 128)
            nc.sync.dma_start(xT_sb[c][:, 0:TQ], xT[sl, 0:TQ])
            nc.sync.dma_start(wq_sb[c][:], wqT[sl, :])
        for c in range(NCC):
            sl = slice(c * 128, (c + 1) * 128)
            nc.sync.dma_start(wk_sb[c][:], wkT[sl, :])
            nc.sync.dma_start(wv_sb[c][:], wvT[sl, :])
        for t in range(1, T // TQ):
            tsl = slice(t * TQ, (t + 1) * TQ)
            for c in range(NCC):
                sl = slice(c * 128, (c + 1) * 128)
                nc.sync.dma_start(xT_sb[c][:, tsl], xT[sl, tsl])
        for m in range(2):
            nc.sync.dma_start(wp_sb[m][:], wpT[m * 128:(m + 1) * 128, :])

        # ---- interleaved emission: projections / attention / out-proj ----
        # The PE executes its queue in order, so emission order controls PE
        # density.  Attention for q-tile j only needs projections up to
        # t=j, so projections for t=j+1 and the out-projection for j-1 are
        # woven between attention groups of j to fill PE idle slots (keeps
        # the HAM clock-gate warm).
        def proj_qk(w_sb, b_sb, dst, m, t):
            tsl = slice(t * TQ, (t + 1) * TQ)
            msl = slice(m * 128, (m + 1) * 128)
            ps = ps_small.tile([128, TQ], f32, tag="ps_small")
            for c in range(NCC):
                nc.tensor.matmul(ps[:], w_sb[c][:, msl], xT_sb[c][:, tsl],
                                 start=(c == 0),
                                 stop=(c == NCC - 1 and not with_qk_bias))
            if with_qk_bias:
                nc.tensor.matmul(ps[:], b_sb[0:1, msl], ones_sb[0:1, :],
                                 start=False, stop=True)
            nc.vector.tensor_copy(dst[m][:, tsl], ps[:])

        def proj_v(tt):
            ttsl = slice(tt * 128, tt * 128 + 128)
            ps = ps_small.tile([128, VW], f32, tag="ps_small")
            for c in range(NCC):
                nc.tensor.matmul(ps[:], xT_sb[c][:, ttsl], wv_sb[c][:],
                                 start=(c == 0), stop=False)
            # always emitted: supplies the ones-columns (+ v bias)
            nc.tensor.matmul(ps[:], ones_sb[0:1, 0:128], bv_sb[:],
                             start=False, stop=True)
            nc.vector.tensor_copy(v_sb[tt][:], ps[:])

        def proj_pieces(t):
            out = []
            for w_sb, b_sb, dst in ((wq_sb, bq_sb, qT_sb), (wk_sb, bk_sb, kT_sb)):
                for m in range(2):
                    out.append(lambda w=w_sb, b=b_sb, d=dst, mm=m:
                               proj_qk(w, b, d, mm, t))
            for tt in range(t * 4, t * 4 + 4):
                out.append(lambda x=tt: proj_v(x))
            return out

        def outproj_piece(tt, do):
            ttsl = slice(tt * 128, (tt + 1) * 128)
            dsl = slice(do * TQ, (do + 1) * TQ)
            ops = ps_small.tile([128, TQ], f32, tag="ps_small")
            for m2 in range(2):
                nc.tensor.matmul(ops[:], yT_sb[m2][:, ttsl],
                                 wp_sb[m2][:, dsl],
                                 start=(m2 == 0), stop=(m2 == 1))
            so = stage.tile([128, TQ], f32, tag="so")
            nc.vector.tensor_copy(so[:], ops[:])
            nc.sync.dma_start(po[ttsl, dsl], so[:])

        def outproj_pieces(j):
            return [lambda t=tt, d=do: outproj_piece(t, d)
                    for tt in range(4 * j, 4 * j + 4) for do in range(2)]

        def s_group(j, h, kcs):
            """Emit S matmuls for a k-chunk pair; return (st_tile, info)."""
            m, pr = h // 2, (h % 2) * 64
            st = ps_st.tile([128, 1024], f32, tag="st")
            info = []
            for i, kc in enumerate(kcs):
                coff = max(0, kc * KC - j * TQ)   # causal column offset
                nc.tensor.matmul(
                    st[:, i * TQ + coff:(i + 1) * TQ],
                    kT_sb[m][pr:pr + 64, kc * KC:(kc + 1) * KC],
                    qT_sb[m][pr:pr + 64, j * TQ + coff:(j + 1) * TQ],
                    start=True, stop=True)
                info.append((i, kc, coff))
            return st, info

        def pv_group(j, h, st, info, yt, nk):
            """exp + triangle mask + PV matmuls for a prepared S group."""
            pt = pt_pool.tile([128, 1024], fmm, tag="pt")
            runs = []
            for i, kc, coff in info:
                lo, hi = i * TQ + coff, (i + 1) * TQ
                if runs and runs[-1][1] == lo:
                    runs[-1][1] = hi
                else:
                    runs.append([lo, hi])
            for lo, hi in runs:
                nc.scalar.activation(pt[:, lo:hi], st[:, lo:hi], Exp, scale=0.125)
            for i, kc, coff in info:
                if kc >= 4 * j:   # diagonal chunk: mask the 128-wide triangle
                    lo = i * TQ + coff
                    nc.vector.tensor_mul(pt[:, lo:lo + KC], pt[:, lo:lo + KC],
                                         tri_sb[:])
            for i, kc, coff in info:
                lo = i * TQ + coff
                nc.tensor.matmul(
                    yt[0:65, coff:TQ] if coff else yt[:],
                    v_sb[kc][:, h * 65:(h + 1) * 65],
                    pt[:, lo:(i + 1) * TQ],
                    start=(kc == 0), stop=(kc == nk - 1))

        def normalize(j, h, yt):
            """yT[h slice, j] = yt[0:64] * broadcast(1/l)."""
            m, pr = h // 2, (h % 2) * 64
            l_sb = norm_pool.tile([1, TQ], fmm, tag="l")
            nc.vector.tensor_copy(l_sb[:], yt[64:65, :])
            bc_ps = ps_small.tile([64, TQ], f32, tag="ps_small")
            nc.tensor.matmul(bc_ps[:], ones_sb[0:1, 0:64], l_sb[:],
                             start=True, stop=True)
            bc_sb = stage.tile([64, TQ], f32, tag="bc")
            nc.vector.reciprocal_approx_fast(bc_sb[:], bc_ps[:])
            nc.vector.tensor_mul(yT_sb[m][pr:pr + 64, j * TQ:(j + 1) * TQ],
                                 yt[0:64, :], bc_sb[:])

        for piece in proj_pieces(0):    # prologue
            piece()

        for j in range(T // TQ):
            nk = 4 * (j + 1)
            groups = []
            for h in range(HLOC):
                for k0 in range(0, nk, 2):
                    groups.append((h, [k for k in (k0, k0 + 1) if k < nk]))
            extras = []
            if j + 1 < T // TQ:
                extras += proj_pieces(j + 1)
            if j == T // TQ - 1:
                # all deferred out-projections: PE filler for the
                # ACT-paced final block (keeps HAM warm)
                for jj in range(T // TQ - 1):
                    extras += outproj_pieces(jj)
            ei = 0           # extras emitted so far
            yts = {}
            pending = None   # (h, st, info) awaiting exp/PV
            done_head = None  # head awaiting normalize
            for gi, (h, kcs) in enumerate(groups):
                if h not in yts:
                    yts[h] = ps_yt.tile([65, TQ], f32, tag="yt",
                                        name=f"yt{j}_{h}")
                st, info = s_group(j, h, kcs)
                if pending is not None:
                    ph, pst, pinfo = pending
                    pv_group(j, ph, pst, pinfo, yts[ph], nk)
                    if ph != h:
                        done_head = ph
                    elif done_head is not None:
                        normalize(j, done_head, yts.pop(done_head))
                        done_head = None
                pending = (h, st, info)
                want = (gi + 1) * len(extras) // len(groups)
                while ei < want:
                    extras[ei]()
                    ei += 1
            ph, pst, pinfo = pending
            pv_group(j, ph, pst, pinfo, yts[ph], nk)
            if done_head is not None:
                normalize(j, done_head, yts.pop(done_head))
            normalize(j, ph, yts.pop(ph))
            while ei < len(extras):
                extras[ei]()
                ei += 1

        for piece in outproj_pieces(T // TQ - 1):   # epilogue
            piece()
    nc.compile()
    return nc


def make_in_maps(x, Wq, bq, Wk, bk, Wv, bv, Wp, bp):
    x = np.asarray(x, np.float32)
    Wq, Wk, Wv, Wp = (np.asarray(w, np.float32) for w in (Wq, Wk, Wv, Wp))
    bq, bk, bv = (np.asarray(b, np.float32) for b in (bq, bk, bv))

    ones = np.ones((1, TQ), np.float32)
    kp = np.arange(KC)[:, None]
    qf = np.arange(KC)[None, :]
    tri = (qf >= kp).astype(np.float32)

    in_maps = []
    for core in range(N_CORES):
        b = core // 4
        hg = core % 4
        rows = slice(hg * CLOC, (hg + 1) * CLOC)
        wv_aug = np.zeros((C, VW), np.float32)
        bv_aug = np.zeros((1, VW), np.float32)
        for h in range(HLOC):
            wsl = slice(hg * CLOC + h * D, hg * CLOC + (h + 1) * D)
            wv_aug[:, h * 65:h * 65 + D] = Wv[wsl, :].T
            bv_aug[0, h * 65:h * 65 + D] = bv[wsl]
            bv_aug[0, h * 65 + D] = 1.0
        in_maps.append({
            "xT": np.ascontiguousarray(x[b].T),
            "wqT": np.ascontiguousarray(Wq[rows, :].T),
            "wkT": np.ascontiguousarray(Wk[rows, :].T),
            "wvT": wv_aug,
            "wpT": np.ascontiguousarray(Wp[:, rows].T),
            "bq": np.ascontiguousarray(bq[rows][None, :]),
            "bk": np.ascontiguousarray(bk[rows][None, :]),
            "bv": bv_aug,
            "ones": ones,
            "tri": tri,
        })
    return in_maps


def kernel(x, Wq, bq, Wk, bk, Wv, bv, Wp, bp):
    from concourse.bass_utils import run_bass_kernel_spmd

    with_qk_bias = bool(np.any(np.asarray(bq)) or np.any(np.asarray(bk)))
    key = ("nc", with_qk_bias)
    if key not in _CACHE:
        _CACHE[key] = build_nc(with_qk_bias)
    nc = _CACHE[key]
    in_maps = make_in_maps(x, Wq, bq, Wk, bk, Wv, bv, Wp, bp)
    res = run_bass_kernel_spmd(nc, in_maps, core_ids=list(range(N_CORES)))
    out = np.zeros((B, T, C), np.float32)
    for core in range(N_CORES):
        out[core // 4] += res.results[core]["po"]
    out += np.asarray(bp, np.float32)[None, None, :]
    return out
